# revision 5
# baseline (speedup 1.0000x reference)
"""MoE gate (DeepSeek-style top-8 router) on 8 Trainium2 cores.

Full-input contract: kernel(x, gate_w) -> (topk_w, topk_idx, aux_loss).

Strategy (data-parallel over tokens, per the sharding hint):
  - Host: transpose each 1024-token shard of x to [7168, 1024] so the
    contraction dim lands on SBUF partitions; fold the 2.5 route scale
    into a replicated wT = (2.5*gate_w).T [7168, 256].
  - Device (per core): logits[t,e] accumulated in PSUM over 56 K-chunks.
    Two matmul modes:
      fp32   -- true-fp32 matmuls (4 PE passes, 4 cyc/row)
      bf16x3 -- split each fp32 operand into bf16 hi + bf16 lo and compute
                xh@wh + xh@wl + xl@wh (3 bf16 passes, 3 cyc/row, ~1.33x
                faster; drops only the ~2^-18 lo*lo term)
    Softmax numerator via ACT Exp (bias = -rowmax, accum_out = denominator),
    top-8 of the *logits* via the DVE max/max_index instructions (descending
    values, lowest-index-first ties -- identical to jax.lax.top_k),
    renormalized top-8 weights, and a running [128,256] score accumulator
    reduced across partitions by a final ones-matmul into the per-expert
    score-sum partial.
  - Host: concat w/idx shards; p = sum(partials)/8192, f from a bincount of
    the indices, aux = 256 * sum(f*p).
"""

import os
import sys

import numpy as np

for _p in ("/opt/trn_rl_repo", "/root/.axon_site/_ro/trn_rl_repo"):
    if os.path.isdir(_p) and _p not in sys.path:
        sys.path.append(_p)

import ml_dtypes  # noqa: E402

import concourse.bass as bass  # noqa: E402
import concourse.tile as tile  # noqa: E402
from concourse import bacc, mybir  # noqa: E402
from concourse.bass_utils import run_bass_kernel_spmd  # noqa: E402

DIM = 7168
N_EXPERTS = 256
TOP_K = 8
ROUTE_SCALE = 2.5
N_TOKENS = 8192
N_CORES = 8
TPC = N_TOKENS // N_CORES          # tokens per core = 1024
KC = DIM // 128                    # contraction chunks = 56
G = 256                            # tokens per x-DMA group
NG = TPC // G                      # groups per core = 4
NB = G // 128                      # 128-token blocks per group = 2
NBLK = TPC // 128                  # blocks per core = 8

F32 = mybir.dt.float32
BF16 = mybir.dt.bfloat16
U32 = mybir.dt.uint32
BF16_NP = ml_dtypes.bfloat16

MODE = os.environ.get("MOE_GATE_MODE", "bf16x3")  # "fp32" | "bf16x3"


def _emit_epilogue(nc, tc, pl, negm, spool, tiny, k8p, w8_d, i8_d, sc_acc, blk):
    """Softmax + top-8 + score accumulation for one 128-token block whose
    logits sit in PSUM tile `pl`."""
    nc.vector.reduce_max(negm[:], pl[:], axis=mybir.AxisListType.X,
                         negate=True)
    sl = spool.tile([128, N_EXPERTS], F32, tag="sl")
    nc.vector.tensor_copy(sl[:], pl[:])
    exps = spool.tile([128, N_EXPERTS], F32, tag="exps")
    den = tiny.tile([128, 1], F32, tag="den")
    nc.scalar.activation(exps[:], pl[:], mybir.ActivationFunctionType.Exp,
                         bias=negm[:], scale=1.0, accum_out=den[:])

    v8 = k8p.tile([128, TOP_K], F32, tag="v8")
    nc.vector.max(v8[:], sl[:])
    i8 = k8p.tile([128, TOP_K], U32, tag="i8")
    nc.vector.max_index(i8[:], v8[:], sl[:])

    e8 = k8p.tile([128, TOP_K], F32, tag="e8")
    s8 = tiny.tile([128, 1], F32, tag="s8")
    nc.scalar.activation(e8[:], v8[:], mybir.ActivationFunctionType.Exp,
                         bias=negm[:], scale=1.0, accum_out=s8[:])
    rs8 = tiny.tile([128, 1], F32, tag="rs8")
    nc.vector.reciprocal(rs8[:], s8[:])
    w8 = k8p.tile([128, TOP_K], F32, tag="w8")
    nc.vector.tensor_scalar_mul(w8[:], e8[:], rs8[:])

    rden = tiny.tile([128, 1], F32, tag="rden")
    nc.vector.reciprocal(rden[:], den[:])
    sc = spool.tile([128, N_EXPERTS], F32, tag="sc")
    nc.vector.tensor_scalar_mul(sc[:], exps[:], rden[:])
    nc.vector.tensor_add(sc_acc[:], sc_acc[:], sc[:])

    nc.sync.dma_start(w8_d[blk * 128:(blk + 1) * 128, :], w8[:])
    nc.sync.dma_start(i8_d[blk * 128:(blk + 1) * 128, :], i8[:])


def build_nc(mode=MODE):
    """Build + compile the per-core Bass program (SPMD: same program on all
    8 cores, different input data)."""
    nc = bacc.Bacc("TRN2", target_bir_lowering=False, debug=False,
                   num_devices=N_CORES)

    w8_d = nc.dram_tensor("w8", [TPC, TOP_K], F32, kind="ExternalOutput")
    i8_d = nc.dram_tensor("i8", [TPC, TOP_K], U32, kind="ExternalOutput")
    pp_d = nc.dram_tensor("pp", [1, N_EXPERTS], F32, kind="ExternalOutput")

    if mode == "fp32":
        xt = nc.dram_tensor("xt", [DIM, TPC], F32, kind="ExternalInput")
        wt = nc.dram_tensor("wt", [DIM, N_EXPERTS], F32, kind="ExternalInput")
        x_drams = [xt.rearrange("(k p) t -> p k t", p=128)]
        w_drams = [wt.rearrange("(k p) e -> p k e", p=128)]
        dt = F32
    elif mode == "bf16x3":
        xh = nc.dram_tensor("xh", [DIM, TPC], BF16, kind="ExternalInput")
        xl = nc.dram_tensor("xl", [DIM, TPC], BF16, kind="ExternalInput")
        wh = nc.dram_tensor("wh", [DIM, N_EXPERTS], BF16, kind="ExternalInput")
        wl = nc.dram_tensor("wl", [DIM, N_EXPERTS], BF16, kind="ExternalInput")
        x_drams = [t.rearrange("(k p) t -> p k t", p=128) for t in (xh, xl)]
        w_drams = [t.rearrange("(k p) e -> p k e", p=128) for t in (wh, wl)]
        dt = BF16
    else:
        raise ValueError(mode)

    with tile.TileContext(nc) as tc:
        with (
            tc.tile_pool(name="wpool", bufs=1) as wpool,
            tc.tile_pool(name="xpool", bufs=2) as xpool,
            tc.tile_pool(name="spool", bufs=3) as spool,
            tc.tile_pool(name="acc", bufs=1) as accp,
            tc.tile_pool(name="tiny", bufs=12) as tiny,
            tc.tile_pool(name="k8", bufs=4) as k8p,
            tc.tile_pool(name="psl", bufs=4, space=bass.MemorySpace.PSUM) as psl,
            tc.tile_pool(name="psp", bufs=1, space=bass.MemorySpace.PSUM) as psp,
        ):
            # per-chunk DMAs so the first matmuls start as soon as chunk 0
            # lands (one monolithic transfer stalls PE ~40us at startup)
            w_sbs = []
            for i, wd in enumerate(w_drams):
                wsb = wpool.tile([128, KC, N_EXPERTS], dt, tag=f"w{i}")
                for k in range(KC):
                    nc.sync.dma_start(wsb[:, k, :], wd[:, k, :])
                w_sbs.append(wsb)

            ones = accp.tile([128, 1], F32)
            nc.vector.memset(ones[:], 1.0)
            sc_acc = accp.tile([128, N_EXPERTS], F32)
            nc.vector.memset(sc_acc[:], 0.0)

            for g in range(NG):
                xgs = []
                for i, xd in enumerate(x_drams):
                    xg = xpool.tile([128, KC, G], dt, tag=f"x{i}")
                    for k in range(KC):
                        nc.sync.dma_start(xg[:, k, :],
                                          xd[:, k, g * G:(g + 1) * G])
                    xgs.append(xg)

                for b in range(NB):
                    blk = g * NB + b
                    ts = slice(b * 128, (b + 1) * 128)
                    pl = psl.tile([128, N_EXPERTS], F32)
                    for k in range(KC):
                        if mode == "fp32":
                            nc.tensor.matmul(pl[:], xgs[0][:, k, ts],
                                             w_sbs[0][:, k, :],
                                             start=(k == 0),
                                             stop=(k == KC - 1))
                        else:
                            # xh@wh + xh@wl + xl@wh (lo*lo dropped)
                            nc.tensor.matmul(pl[:], xgs[0][:, k, ts],
                                             w_sbs[0][:, k, :],
                                             start=(k == 0), stop=False)
                            nc.tensor.matmul(pl[:], xgs[0][:, k, ts],
                                             w_sbs[1][:, k, :],
                                             start=False, stop=False)
                            nc.tensor.matmul(pl[:], xgs[1][:, k, ts],
                                             w_sbs[0][:, k, :],
                                             start=False, stop=(k == KC - 1))

                    negm = tiny.tile([128, 1], F32, tag="negm")
                    _emit_epilogue(nc, tc, pl, negm, spool, tiny, k8p,
                                   w8_d, i8_d, sc_acc, blk)

            # cross-partition (token) reduce of the score accumulator
            pp_ps = psp.tile([128, N_EXPERTS], F32)
            nc.tensor.matmul(pp_ps[:1, :], ones[:], sc_acc[:],
                             start=True, stop=True)
            pp_sb = accp.tile([1, N_EXPERTS], F32)
            nc.vector.tensor_copy(pp_sb[:], pp_ps[:1, :])
            nc.sync.dma_start(pp_d[:], pp_sb[:])

    nc.compile()
    return nc


_NC = {}


def _get_nc(mode=MODE):
    if mode not in _NC:
        _NC[mode] = build_nc(mode)
    return _NC[mode]


def make_in_maps(x, gate_w, mode=MODE):
    x = np.asarray(x, dtype=np.float32)
    gate_w = np.asarray(gate_w, dtype=np.float32)
    assert x.shape == (N_TOKENS, DIM), x.shape
    assert gate_w.shape == (N_EXPERTS, DIM), gate_w.shape
    wt = np.ascontiguousarray((gate_w * np.float32(ROUTE_SCALE)).T)
    in_maps = []
    if mode == "fp32":
        for c in range(N_CORES):
            shard = x[c * TPC:(c + 1) * TPC, :]
            in_maps.append({"xt": np.ascontiguousarray(shard.T), "wt": wt})
    else:
        wh = wt.astype(BF16_NP)
        wl = (wt - wh.astype(np.float32)).astype(BF16_NP)
        for c in range(N_CORES):
            xt = np.ascontiguousarray(x[c * TPC:(c + 1) * TPC, :].T)
            xh = xt.astype(BF16_NP)
            xlo = (xt - xh.astype(np.float32)).astype(BF16_NP)
            in_maps.append({"xh": xh, "xl": xlo, "wh": wh, "wl": wl})
    return in_maps


def combine_results(results):
    topk_w = np.concatenate([r["w8"] for r in results], axis=0)
    topk_idx = np.concatenate([r["i8"] for r in results], axis=0).astype(np.int32)
    p_sum = np.sum(np.stack([r["pp"][0] for r in results]), axis=0,
                   dtype=np.float32)
    p = p_sum / np.float32(N_TOKENS)
    f = (np.bincount(topk_idx.ravel(), minlength=N_EXPERTS)
         .astype(np.float32) / np.float32(N_TOKENS))
    aux_loss = np.float32(np.sum(f * p, dtype=np.float32) * np.float32(N_EXPERTS))
    return topk_w.astype(np.float32), topk_idx, aux_loss


def kernel(x, gate_w):
    nc = _get_nc()
    in_maps = make_in_maps(x, gate_w)
    res = run_bass_kernel_spmd(nc, in_maps, list(range(N_CORES)))
    return combine_results(res.results)


# revision 10
# speedup vs baseline: 1.9687x; 1.9687x over previous
"""MoE gate (DeepSeek-style top-8 router) on 8 Trainium2 cores.

Full-input contract: kernel(x, gate_w) -> (topk_w, topk_idx, aux_loss).

Strategy (data-parallel over tokens, per the sharding hint):
  - Host: transpose each 1024-token shard of x to [7168, 1024] so the
    contraction dim lands on SBUF partitions; fold the 2.5 route scale
    into a replicated wT = (2.5*gate_w).T [7168, 256].
  - Device (per core): logits[t,e] accumulated in PSUM over 56 K-chunks.
    Two matmul modes:
      fp32   -- true-fp32 matmuls (4 PE passes, 4 cyc/row)
      bf16x3 -- split each fp32 operand into bf16 hi + bf16 lo and compute
                xh@wh + xh@wl + xl@wh (3 bf16 passes, 3 cyc/row, ~1.33x
                faster; drops only the ~2^-18 lo*lo term)
    Softmax numerator via ACT Exp (bias = -rowmax, accum_out = denominator),
    top-8 of the *logits* via the DVE max/max_index instructions (descending
    values, lowest-index-first ties -- identical to jax.lax.top_k),
    renormalized top-8 weights, and a running [128,256] score accumulator
    reduced across partitions by a final ones-matmul into the per-expert
    score-sum partial.
  - Host: concat w/idx shards; p = sum(partials)/8192, f from a bincount of
    the indices, aux = 256 * sum(f*p).
"""

import os
import sys

import numpy as np

for _p in ("/opt/trn_rl_repo", "/root/.axon_site/_ro/trn_rl_repo"):
    if os.path.isdir(_p) and _p not in sys.path:
        sys.path.append(_p)

import ml_dtypes  # noqa: E402

import concourse.bass as bass  # noqa: E402
import concourse.tile as tile  # noqa: E402
from concourse import bacc, mybir  # noqa: E402
from concourse.bass_utils import run_bass_kernel_spmd  # noqa: E402

DIM = 7168
N_EXPERTS = 256
TOP_K = 8
ROUTE_SCALE = 2.5
N_TOKENS = 8192
N_CORES = 8
TPC = N_TOKENS // N_CORES          # tokens per core = 1024
KC = DIM // 128                    # contraction chunks = 56
G = 256                            # tokens per x-DMA group
NG = TPC // G                      # groups per core = 4
NB = G // 128                      # 128-token blocks per group = 2
NBLK = TPC // 128                  # blocks per core = 8

F32 = mybir.dt.float32
BF16 = mybir.dt.bfloat16
U32 = mybir.dt.uint32
BF16_NP = ml_dtypes.bfloat16

MODE = os.environ.get("MOE_GATE_MODE", "bf16x3")  # "fp32" | "bf16x3"


def _emit_epilogue(nc, tc, pl, negm, spool, tiny, k8p, w8_d, i8_d, sc_acc, blk):
    """Softmax + top-8 + score accumulation for one 128-token block whose
    logits sit in PSUM tile `pl`."""
    nc.vector.reduce_max(negm[:], pl[:], axis=mybir.AxisListType.X,
                         negate=True)
    sl = spool.tile([128, N_EXPERTS], F32, tag="sl")
    nc.vector.tensor_copy(sl[:], pl[:])
    exps = spool.tile([128, N_EXPERTS], F32, tag="exps")
    den = tiny.tile([128, 1], F32, tag="den")
    nc.scalar.activation(exps[:], pl[:], mybir.ActivationFunctionType.Exp,
                         bias=negm[:], scale=1.0, accum_out=den[:])

    v8 = k8p.tile([128, TOP_K], F32, tag="v8")
    nc.vector.max(v8[:], sl[:])
    i8 = k8p.tile([128, TOP_K], U32, tag="i8")
    nc.vector.max_index(i8[:], v8[:], sl[:])

    e8 = k8p.tile([128, TOP_K], F32, tag="e8")
    s8 = tiny.tile([128, 1], F32, tag="s8")
    nc.scalar.activation(e8[:], v8[:], mybir.ActivationFunctionType.Exp,
                         bias=negm[:], scale=1.0, accum_out=s8[:])
    rs8 = tiny.tile([128, 1], F32, tag="rs8")
    nc.vector.reciprocal(rs8[:], s8[:])
    w8 = k8p.tile([128, TOP_K], F32, tag="w8")
    nc.vector.tensor_scalar_mul(w8[:], e8[:], rs8[:])

    rden = tiny.tile([128, 1], F32, tag="rden")
    nc.vector.reciprocal(rden[:], den[:])
    sc = spool.tile([128, N_EXPERTS], F32, tag="sc")
    nc.vector.tensor_scalar_mul(sc[:], exps[:], rden[:])
    nc.vector.tensor_add(sc_acc[:], sc_acc[:], sc[:])

    nc.sync.dma_start(w8_d[blk * 128:(blk + 1) * 128, :], w8[:])
    nc.sync.dma_start(i8_d[blk * 128:(blk + 1) * 128, :], i8[:])


def build_nc(mode=MODE):
    """Build + compile the per-core Bass program (SPMD: same program on all
    8 cores, different input data)."""
    nc = bacc.Bacc("TRN2", target_bir_lowering=False, debug=False,
                   num_devices=N_CORES)

    w8_d = nc.dram_tensor("w8", [TPC, TOP_K], F32, kind="ExternalOutput")
    i8_d = nc.dram_tensor("i8", [TPC, TOP_K], U32, kind="ExternalOutput")
    pp_d = nc.dram_tensor("pp", [1, N_EXPERTS], F32, kind="ExternalOutput")

    # Inputs are host-packed to put the partition dim first so every DMA
    # moves multi-KB contiguous runs per partition line (512B-1KB lines made
    # the transfer descriptor-bound: ~72K descriptors @ ~85ns/queue starved
    # the PE).  x_packed[p, g, k, t] = x.T[k*128+p, g*G+t].
    if mode == "fp32":
        x_drams = [nc.dram_tensor("xp", [128, NG, KC, G], F32,
                                  kind="ExternalInput")]
        w_drams = [nc.dram_tensor("wp", [128, KC, N_EXPERTS], F32,
                                  kind="ExternalInput")]
        dt = F32
    elif mode == "bf16x3":
        x_drams = [nc.dram_tensor(n, [128, NG, KC, G], BF16,
                                  kind="ExternalInput") for n in ("xph", "xpl")]
        w_drams = [nc.dram_tensor(n, [128, KC, N_EXPERTS], BF16,
                                  kind="ExternalInput") for n in ("wph", "wpl")]
        dt = BF16
    else:
        raise ValueError(mode)

    with tile.TileContext(nc) as tc:
        with (
            tc.tile_pool(name="wpool", bufs=1) as wpool,
            tc.tile_pool(name="xpool", bufs=2) as xpool,
            tc.tile_pool(name="spool", bufs=3) as spool,
            tc.tile_pool(name="acc", bufs=1) as accp,
            tc.tile_pool(name="tiny", bufs=12) as tiny,
            tc.tile_pool(name="k8", bufs=4) as k8p,
            tc.tile_pool(name="psl", bufs=4, space=bass.MemorySpace.PSUM) as psl,
            tc.tile_pool(name="psp", bufs=1, space=bass.MemorySpace.PSUM) as psp,
        ):
            # sub-DMAs of 7 k-chunks each: first matmuls start after ~2 of
            # them, while keeping lines contiguous (7 chunks x E x dt per
            # partition per transfer)
            SUB = 7
            w_sbs = []
            for i, wd in enumerate(w_drams):
                wsb = wpool.tile([128, KC, N_EXPERTS], dt, tag=f"w{i}")
                w_sbs.append(wsb)
            for k0 in range(0, KC, SUB):
                for wsb, wd in zip(w_sbs, w_drams):
                    nc.sync.dma_start(wsb[:, k0:k0 + SUB, :],
                                      wd[:, k0:k0 + SUB, :])

            ones = accp.tile([128, 1], F32)
            nc.vector.memset(ones[:], 1.0)
            sc_acc = accp.tile([128, N_EXPERTS], F32)
            nc.vector.memset(sc_acc[:], 0.0)

            for g in range(NG):
                xgs = []
                for i, xd in enumerate(x_drams):
                    xgs.append(xpool.tile([128, KC, G], dt, tag=f"x{i}",
                                          name=f"xg{i}"))
                for k0 in range(0, KC, SUB):
                    for xg, xd in zip(xgs, x_drams):
                        nc.sync.dma_start(xg[:, k0:k0 + SUB, :],
                                          xd[:, g, k0:k0 + SUB, :])

                for b in range(NB):
                    blk = g * NB + b
                    ts = slice(b * 128, (b + 1) * 128)
                    pl = psl.tile([128, N_EXPERTS], F32)
                    for k in range(KC):
                        if mode == "fp32":
                            nc.tensor.matmul(pl[:], xgs[0][:, k, ts],
                                             w_sbs[0][:, k, :],
                                             start=(k == 0),
                                             stop=(k == KC - 1))
                        else:
                            # xh@wh + xh@wl + xl@wh (lo*lo dropped)
                            nc.tensor.matmul(pl[:], xgs[0][:, k, ts],
                                             w_sbs[0][:, k, :],
                                             start=(k == 0), stop=False)
                            nc.tensor.matmul(pl[:], xgs[0][:, k, ts],
                                             w_sbs[1][:, k, :],
                                             start=False, stop=False)
                            nc.tensor.matmul(pl[:], xgs[1][:, k, ts],
                                             w_sbs[0][:, k, :],
                                             start=False, stop=(k == KC - 1))

                    negm = tiny.tile([128, 1], F32, tag="negm")
                    _emit_epilogue(nc, tc, pl, negm, spool, tiny, k8p,
                                   w8_d, i8_d, sc_acc, blk)

            # cross-partition (token) reduce of the score accumulator
            pp_ps = psp.tile([128, N_EXPERTS], F32)
            nc.tensor.matmul(pp_ps[:1, :], ones[:], sc_acc[:],
                             start=True, stop=True)
            pp_sb = accp.tile([1, N_EXPERTS], F32)
            nc.vector.tensor_copy(pp_sb[:], pp_ps[:1, :])
            nc.sync.dma_start(pp_d[:], pp_sb[:])

    nc.compile()
    return nc


_NC = {}


def _get_nc(mode=MODE):
    if mode not in _NC:
        _NC[mode] = build_nc(mode)
    return _NC[mode]


def _pack_x(shard):
    """[TPC, DIM] -> [128, NG, KC, G] with x_packed[p,g,k,t] = shard[g*G+t, k*128+p]."""
    return np.ascontiguousarray(
        shard.reshape(NG, G, KC, 128).transpose(3, 0, 2, 1))


def _pack_w(wt):
    """[DIM, E] -> [128, KC, E]."""
    return np.ascontiguousarray(
        wt.reshape(KC, 128, N_EXPERTS).transpose(1, 0, 2))


def make_in_maps(x, gate_w, mode=MODE):
    x = np.asarray(x, dtype=np.float32)
    gate_w = np.asarray(gate_w, dtype=np.float32)
    assert x.shape == (N_TOKENS, DIM), x.shape
    assert gate_w.shape == (N_EXPERTS, DIM), gate_w.shape
    wt = (gate_w * np.float32(ROUTE_SCALE)).T  # [DIM, E]
    in_maps = []
    if mode == "fp32":
        wp = _pack_w(wt)
        for c in range(N_CORES):
            xp = _pack_x(x[c * TPC:(c + 1) * TPC, :])
            in_maps.append({"xp": xp, "wp": wp})
    else:
        wp32 = _pack_w(wt)
        wh = wp32.astype(BF16_NP)
        wl = (wp32 - wh.astype(np.float32)).astype(BF16_NP)
        for c in range(N_CORES):
            xp32 = _pack_x(x[c * TPC:(c + 1) * TPC, :])
            xh = xp32.astype(BF16_NP)
            xlo = (xp32 - xh.astype(np.float32)).astype(BF16_NP)
            in_maps.append({"xph": xh, "xpl": xlo, "wph": wh, "wpl": wl})
    return in_maps


def combine_results(results):
    topk_w = np.concatenate([r["w8"] for r in results], axis=0)
    topk_idx = np.concatenate([r["i8"] for r in results], axis=0).astype(np.int32)
    p_sum = np.sum(np.stack([r["pp"][0] for r in results]), axis=0,
                   dtype=np.float32)
    p = p_sum / np.float32(N_TOKENS)
    f = (np.bincount(topk_idx.ravel(), minlength=N_EXPERTS)
         .astype(np.float32) / np.float32(N_TOKENS))
    aux_loss = np.float32(np.sum(f * p, dtype=np.float32) * np.float32(N_EXPERTS))
    return topk_w.astype(np.float32), topk_idx, aux_loss


def kernel(x, gate_w):
    nc = _get_nc()
    in_maps = make_in_maps(x, gate_w)
    res = run_bass_kernel_spmd(nc, in_maps, list(range(N_CORES)))
    return combine_results(res.results)


# revision 12
# speedup vs baseline: 2.3133x; 1.1750x over previous
"""MoE gate (DeepSeek-style top-8 router) on 8 Trainium2 cores.

Full-input contract: kernel(x, gate_w) -> (topk_w, topk_idx, aux_loss).

Strategy (data-parallel over tokens, per the sharding hint):
  - Host: transpose each 1024-token shard of x to [7168, 1024] so the
    contraction dim lands on SBUF partitions; fold the 2.5 route scale
    into a replicated wT = (2.5*gate_w).T [7168, 256].
  - Device (per core): logits[t,e] accumulated in PSUM over 56 K-chunks.
    Two matmul modes:
      fp32   -- true-fp32 matmuls (4 PE passes, 4 cyc/row)
      bf16x3 -- split each fp32 operand into bf16 hi + bf16 lo and compute
                xh@wh + xh@wl + xl@wh (3 bf16 passes, 3 cyc/row, ~1.33x
                faster; drops only the ~2^-18 lo*lo term)
    Softmax numerator via ACT Exp (bias = -rowmax, accum_out = denominator),
    top-8 of the *logits* via the DVE max/max_index instructions (descending
    values, lowest-index-first ties -- identical to jax.lax.top_k),
    renormalized top-8 weights, and a running [128,256] score accumulator
    reduced across partitions by a final ones-matmul into the per-expert
    score-sum partial.
  - Host: concat w/idx shards; p = sum(partials)/8192, f from a bincount of
    the indices, aux = 256 * sum(f*p).
"""

import os
import sys

import numpy as np

for _p in ("/opt/trn_rl_repo", "/root/.axon_site/_ro/trn_rl_repo"):
    if os.path.isdir(_p) and _p not in sys.path:
        sys.path.append(_p)

import ml_dtypes  # noqa: E402

import concourse.bass as bass  # noqa: E402
import concourse.tile as tile  # noqa: E402
from concourse import bacc, mybir  # noqa: E402
from concourse.bass_utils import run_bass_kernel_spmd  # noqa: E402

DIM = 7168
N_EXPERTS = 256
TOP_K = 8
ROUTE_SCALE = 2.5
N_TOKENS = 8192
N_CORES = 8
TPC = N_TOKENS // N_CORES          # tokens per core = 1024
KC = DIM // 128                    # contraction chunks = 56
G = 256                            # tokens per x-DMA group
NG = TPC // G                      # groups per core = 4
NB = G // 128                      # 128-token blocks per group = 2
NBLK = TPC // 128                  # blocks per core = 8

F32 = mybir.dt.float32
BF16 = mybir.dt.bfloat16
U32 = mybir.dt.uint32
BF16_NP = ml_dtypes.bfloat16

MODE = os.environ.get("MOE_GATE_MODE", "bf16x3")  # "fp32" | "bf16x3"


def _emit_epilogue(nc, tc, pl, negm, spool, tiny, k8p, w8_d, i8_d, sc_acc, blk):
    """Softmax + top-8 + score accumulation for one 128-token block whose
    logits sit in PSUM tile `pl`."""
    nc.vector.reduce_max(negm[:], pl[:], axis=mybir.AxisListType.X,
                         negate=True)
    sl = spool.tile([128, N_EXPERTS], F32, tag="sl")
    nc.vector.tensor_copy(sl[:], pl[:])
    exps = spool.tile([128, N_EXPERTS], F32, tag="exps")
    den = tiny.tile([128, 1], F32, tag="den")
    nc.scalar.activation(exps[:], pl[:], mybir.ActivationFunctionType.Exp,
                         bias=negm[:], scale=1.0, accum_out=den[:])

    v8 = k8p.tile([128, TOP_K], F32, tag="v8")
    nc.vector.max(v8[:], sl[:])
    i8 = k8p.tile([128, TOP_K], U32, tag="i8")
    nc.vector.max_index(i8[:], v8[:], sl[:])

    e8 = k8p.tile([128, TOP_K], F32, tag="e8")
    s8 = tiny.tile([128, 1], F32, tag="s8")
    nc.scalar.activation(e8[:], v8[:], mybir.ActivationFunctionType.Exp,
                         bias=negm[:], scale=1.0, accum_out=s8[:])
    rs8 = tiny.tile([128, 1], F32, tag="rs8")
    nc.vector.reciprocal(rs8[:], s8[:])
    w8 = k8p.tile([128, TOP_K], F32, tag="w8")
    nc.vector.tensor_scalar_mul(w8[:], e8[:], rs8[:])

    rden = tiny.tile([128, 1], F32, tag="rden")
    nc.vector.reciprocal(rden[:], den[:])
    sc = spool.tile([128, N_EXPERTS], F32, tag="sc")
    nc.vector.tensor_scalar_mul(sc[:], exps[:], rden[:])
    nc.vector.tensor_add(sc_acc[:], sc_acc[:], sc[:])

    nc.sync.dma_start(w8_d[blk * 128:(blk + 1) * 128, :], w8[:])
    nc.sync.dma_start(i8_d[blk * 128:(blk + 1) * 128, :], i8[:])


def build_nc(mode=MODE):
    """Build + compile the per-core Bass program (SPMD: same program on all
    8 cores, different input data)."""
    nc = bacc.Bacc("TRN2", target_bir_lowering=False, debug=False,
                   num_devices=N_CORES)

    w8_d = nc.dram_tensor("w8", [TPC, TOP_K], F32, kind="ExternalOutput")
    i8_d = nc.dram_tensor("i8", [TPC, TOP_K], U32, kind="ExternalOutput")
    pp_d = nc.dram_tensor("pp", [1, N_EXPERTS], F32, kind="ExternalOutput")

    # Inputs are host-packed to put the partition dim first so every DMA
    # moves multi-KB contiguous runs per partition line (512B-1KB lines made
    # the transfer descriptor-bound: ~72K descriptors @ ~85ns/queue starved
    # the PE).  x_packed[p, g, k, t] = x.T[k*128+p, g*G+t].
    if mode == "fp32":
        x_drams = [nc.dram_tensor("xp", [128, NG, KC, G], F32,
                                  kind="ExternalInput")]
        w_drams = [nc.dram_tensor("wp", [128, KC, N_EXPERTS], F32,
                                  kind="ExternalInput")]
        dt = F32
    elif mode == "bf16x3":
        x_drams = [nc.dram_tensor(n, [128, NG, KC, G], BF16,
                                  kind="ExternalInput") for n in ("xph", "xpl")]
        w_drams = [nc.dram_tensor(n, [128, KC, N_EXPERTS], BF16,
                                  kind="ExternalInput") for n in ("wph", "wpl")]
        dt = BF16
    else:
        raise ValueError(mode)

    with tile.TileContext(nc) as tc:
        with (
            tc.tile_pool(name="wpool", bufs=1) as wpool,
            tc.tile_pool(name="xpool", bufs=2) as xpool,
            tc.tile_pool(name="spool", bufs=3) as spool,
            tc.tile_pool(name="acc", bufs=1) as accp,
            tc.tile_pool(name="tiny", bufs=12) as tiny,
            tc.tile_pool(name="k8", bufs=4) as k8p,
            tc.tile_pool(name="psl", bufs=4, space=bass.MemorySpace.PSUM) as psl,
            tc.tile_pool(name="psp", bufs=1, space=bass.MemorySpace.PSUM) as psp,
        ):
            # sub-DMAs of 7 k-chunks each: first matmuls start after ~2 of
            # them, while keeping lines contiguous (7 chunks x E x dt per
            # partition per transfer).  Descriptor generation on the SP
            # sequencer is ~0.9us per dma_start, so the issue ORDER matters:
            # interleave the group-0 x chunks with the w chunks so the k=0
            # matmuls have data after ~4 descriptor-gens instead of ~18.
            SUB = 7
            w_sbs = []
            for i, wd in enumerate(w_drams):
                wsb = wpool.tile([128, KC, N_EXPERTS], dt, tag=f"w{i}")
                w_sbs.append(wsb)
            xg0s = [xpool.tile([128, KC, G], dt, tag=f"x{i}", name=f"xg{i}")
                    for i in range(len(x_drams))]
            for k0 in range(0, KC, SUB):
                for wsb, wd in zip(w_sbs, w_drams):
                    nc.sync.dma_start(wsb[:, k0:k0 + SUB, :],
                                      wd[:, k0:k0 + SUB, :])
                for xg, xd in zip(xg0s, x_drams):
                    nc.sync.dma_start(xg[:, k0:k0 + SUB, :],
                                      xd[:, 0, k0:k0 + SUB, :])

            ones = accp.tile([128, 1], F32)
            nc.vector.memset(ones[:], 1.0)
            sc_acc = accp.tile([128, N_EXPERTS], F32)
            nc.vector.memset(sc_acc[:], 0.0)

            for g in range(NG):
                if g == 0:
                    xgs = xg0s
                else:
                    xgs = [xpool.tile([128, KC, G], dt, tag=f"x{i}",
                                      name=f"xg{i}")
                           for i in range(len(x_drams))]
                    for k0 in range(0, KC, SUB):
                        for xg, xd in zip(xgs, x_drams):
                            nc.sync.dma_start(xg[:, k0:k0 + SUB, :],
                                              xd[:, g, k0:k0 + SUB, :])

                for b in range(NB):
                    blk = g * NB + b
                    ts = slice(b * 128, (b + 1) * 128)
                    pl = psl.tile([128, N_EXPERTS], F32)
                    for k in range(KC):
                        if mode == "fp32":
                            nc.tensor.matmul(pl[:], xgs[0][:, k, ts],
                                             w_sbs[0][:, k, :],
                                             start=(k == 0),
                                             stop=(k == KC - 1))
                        else:
                            # xh@wh + xh@wl + xl@wh (lo*lo dropped)
                            nc.tensor.matmul(pl[:], xgs[0][:, k, ts],
                                             w_sbs[0][:, k, :],
                                             start=(k == 0), stop=False)
                            nc.tensor.matmul(pl[:], xgs[0][:, k, ts],
                                             w_sbs[1][:, k, :],
                                             start=False, stop=False)
                            nc.tensor.matmul(pl[:], xgs[1][:, k, ts],
                                             w_sbs[0][:, k, :],
                                             start=False, stop=(k == KC - 1))

                    negm = tiny.tile([128, 1], F32, tag="negm")
                    _emit_epilogue(nc, tc, pl, negm, spool, tiny, k8p,
                                   w8_d, i8_d, sc_acc, blk)

            # cross-partition (token) reduce of the score accumulator
            pp_ps = psp.tile([128, N_EXPERTS], F32)
            nc.tensor.matmul(pp_ps[:1, :], ones[:], sc_acc[:],
                             start=True, stop=True)
            pp_sb = accp.tile([1, N_EXPERTS], F32)
            nc.vector.tensor_copy(pp_sb[:], pp_ps[:1, :])
            nc.sync.dma_start(pp_d[:], pp_sb[:])

    nc.compile()
    return nc


_NC = {}


def _get_nc(mode=MODE):
    if mode not in _NC:
        _NC[mode] = build_nc(mode)
    return _NC[mode]


def _pack_x(shard):
    """[TPC, DIM] -> [128, NG, KC, G] with x_packed[p,g,k,t] = shard[g*G+t, k*128+p]."""
    return np.ascontiguousarray(
        shard.reshape(NG, G, KC, 128).transpose(3, 0, 2, 1))


def _pack_w(wt):
    """[DIM, E] -> [128, KC, E]."""
    return np.ascontiguousarray(
        wt.reshape(KC, 128, N_EXPERTS).transpose(1, 0, 2))


def make_in_maps(x, gate_w, mode=MODE):
    x = np.asarray(x, dtype=np.float32)
    gate_w = np.asarray(gate_w, dtype=np.float32)
    assert x.shape == (N_TOKENS, DIM), x.shape
    assert gate_w.shape == (N_EXPERTS, DIM), gate_w.shape
    wt = (gate_w * np.float32(ROUTE_SCALE)).T  # [DIM, E]
    in_maps = []
    if mode == "fp32":
        wp = _pack_w(wt)
        for c in range(N_CORES):
            xp = _pack_x(x[c * TPC:(c + 1) * TPC, :])
            in_maps.append({"xp": xp, "wp": wp})
    else:
        wp32 = _pack_w(wt)
        wh = wp32.astype(BF16_NP)
        wl = (wp32 - wh.astype(np.float32)).astype(BF16_NP)
        for c in range(N_CORES):
            xp32 = _pack_x(x[c * TPC:(c + 1) * TPC, :])
            xh = xp32.astype(BF16_NP)
            xlo = (xp32 - xh.astype(np.float32)).astype(BF16_NP)
            in_maps.append({"xph": xh, "xpl": xlo, "wph": wh, "wpl": wl})
    return in_maps


def combine_results(results):
    topk_w = np.concatenate([r["w8"] for r in results], axis=0)
    topk_idx = np.concatenate([r["i8"] for r in results], axis=0).astype(np.int32)
    p_sum = np.sum(np.stack([r["pp"][0] for r in results]), axis=0,
                   dtype=np.float32)
    p = p_sum / np.float32(N_TOKENS)
    f = (np.bincount(topk_idx.ravel(), minlength=N_EXPERTS)
         .astype(np.float32) / np.float32(N_TOKENS))
    aux_loss = np.float32(np.sum(f * p, dtype=np.float32) * np.float32(N_EXPERTS))
    return topk_w.astype(np.float32), topk_idx, aux_loss


def kernel(x, gate_w):
    nc = _get_nc()
    in_maps = make_in_maps(x, gate_w)
    res = run_bass_kernel_spmd(nc, in_maps, list(range(N_CORES)))
    return combine_results(res.results)
